# revision 18
# baseline (speedup 1.0000x reference)
"""BotRGCN on 8 TRN2 NeuronCores (Bass/Tile SPMD kernel).

Strategy (graph/data parallel, per the sharding hint):
  - Nodes are sharded across 8 cores (12500 nodes/core); edges are grouped by
    destination core/window-pair; the small 128-dim weights are replicated.
  - Activations live TRANSPOSED on-chip: [feat(128 partitions), nodes(free)],
    so every matmul contracts the partition dim with zero transposes.
  - Per RGCN layer: each core all-gathers the node features (natural row
    layout) into a local HBM table, gathers x[src] rows for its local edges
    with dma_gather (table split in 4 quarters so indices fit int16), and
    scatter-adds them into per-window-pair accumulators with one-hot matmuls
    on the tensor engine:
        aggT[feat, 512] += gathered[edges, feat].T @ M[edges, 512]
    where M[e, (win&1)*256 + rel*128 + (dst%128)] = 1/cnt(dst, rel)
    (mean weights folded into the one-hot).
  - The relation transform is then W_r.T @ meanT (weights are naturally [K, M]
    as lhsT), plus Wroot.T @ xT and bias.

The module is a single SPMD program: the instruction stream is identical on
all cores (uniform chunk-slot structure per (window-pair, quarter), padded
with row-0 gathers and -1 one-hot keys); all per-core variation is in data.
"""

import math
from contextlib import ExitStack

import numpy as np

import concourse.bacc as bacc
import concourse.bass as bass
import concourse.mybir as mybir
import concourse.tile as tile
from concourse import bass_utils
from concourse.masks import make_identity

F32 = mybir.dt.float32
BF16 = mybir.dt.bfloat16
I16 = mybir.dt.int16
SLOPE = 0.01
N_CORES = 8
NQ = 4  # gather-table quarters (int16 index range)


# ---------------------------------------------------------------------------
# Host-side preprocessing: shard + sort edges, build gather/one-hot metadata
# ---------------------------------------------------------------------------

def _preprocess(edge_index, edge_type, n_nodes, n_cores=N_CORES, sgp_max=8):
    src = np.asarray(edge_index[0], dtype=np.int64)
    dst = np.asarray(edge_index[1], dtype=np.int64)
    et = np.asarray(edge_type, dtype=np.int64)
    E = src.shape[0]
    npc = n_nodes // n_cores
    assert npc * n_cores == n_nodes
    nw = (npc + 127) // 128        # windows (128 dst nodes) per core
    npairs = (nw + 1) // 2         # window pairs (512-wide one-hot)
    qrows = (n_nodes + NQ - 1) // NQ

    # mean-aggregation weights: 1 / count(dst, rel), computed globally
    seg = dst * 2 + et
    cnt = np.bincount(seg, minlength=2 * n_nodes)
    w_edge = (1.0 / np.maximum(cnt[seg], 1)).astype(np.float32)

    core = dst // npc
    dstl = dst - core * npc
    win = dstl >> 7
    pair = win >> 1
    q = src // qrows
    key = ((win & 1) * 256 + et * 128 + (dstl & 127)).astype(np.float32)

    # group edges by (core, pair, q)
    gid = (core * npairs + pair) * NQ + q
    order = np.argsort(gid, kind="stable")
    gid_s = gid[order]
    counts = np.bincount(gid_s, minlength=n_cores * npairs * NQ)
    starts = np.zeros(counts.size + 1, dtype=np.int64)
    np.cumsum(counts, out=starts[1:])
    pos = np.arange(E, dtype=np.int64) - starts[gid_s]

    kq = max(1, int(math.ceil(counts.max() / 128)))  # chunk slots per (pair,q)
    # dma_gather is limited to ~1024 descriptors per call (SWDGE ring);
    # keep per-call num_idxs = sgp*kq*128 <= 1024
    sgp = min(max(1, 8 // kq), npairs, sgp_max)       # pairs per supergroup
    groups = (npairs + sgp - 1) // sgp
    sg_sizes = [min(sgp, npairs - g * sgp) for g in range(groups)]
    slots = npairs * NQ * kq

    # slot numbering is gather-call-major: for supergroup g with S pairs,
    # call (g, q) covers slots [base(g) + q*S*kq, base(g) + (q+1)*S*kq)
    sg_base = np.zeros(groups, dtype=np.int64)
    acc = 0
    for g in range(groups):
        sg_base[g] = acc
        acc += sg_sizes[g] * NQ * kq
    assert acc == slots
    sgsz = np.array(sg_sizes, dtype=np.int64)

    # everything below is in sorted-edge order (aligned with `pos`)
    q_s = q[order]
    pair_s = pair[order]
    sg_s = pair_s // sgp
    pl_s = pair_s - sg_s * sgp
    e_slot = sg_base[sg_s] + (q_s * sgsz[sg_s] + pl_s) * kq + (pos >> 7)
    e_p = (pos & 127).astype(np.int64)

    gidx = np.zeros((n_cores, slots * 128), dtype=np.int16)
    keym = np.full((n_cores, 128, slots), -1.0, dtype=np.float32)
    wgtm = np.zeros((n_cores, 128, slots), dtype=np.float32)

    e_core = core[order]
    gidx[e_core, e_slot * 128 + e_p] = (src[order] - q_s * qrows
                                        ).astype(np.int16)
    keym[e_core, e_p, e_slot] = key[order]
    wgtm[e_core, e_p, e_slot] = w_edge[order]

    # wrap indices: position i -> [i%16, i//16], replicated to 128 partitions
    idx16 = np.ascontiguousarray(
        gidx.reshape(n_cores, slots * 8, 16).transpose(0, 2, 1))
    idx16 = np.tile(idx16, (1, 8, 1))  # [n_cores, 128, slots*8]

    return dict(
        npc=npc, nw=nw, npairs=npairs, qrows=qrows, kq=kq, sgp=sgp,
        groups=groups, sg_sizes=sg_sizes, slots=slots,
        idx16=idx16, keym=keym, wgtm=wgtm,
    )


# ---------------------------------------------------------------------------
# Device kernel builder (one SPMD module for all cores)
# ---------------------------------------------------------------------------

def _build_module(N, T, prep, n_cores=N_CORES, gather_dtype="float32",
                  debug_outputs=False, profile_single_core=False):
    D = 128
    KT = T // 128
    assert KT * 128 == T
    npc = prep["npc"]
    nw = prep["nw"]
    npairs = prep["npairs"]
    qrows = prep["qrows"]
    kq = prep["kq"]
    groups = prep["groups"]
    sg_sizes = prep["sg_sizes"]
    slots = prep["slots"]
    npad = nw * 128
    TILE_W = 512
    NT = (npc + TILE_W - 1) // TILE_W
    GDT = F32 if gather_dtype == "float32" else BF16

    if profile_single_core:
        n_cores = 1
    nc = bacc.Bacc("TRN2", target_bir_lowering=False, debug=False,
                   enable_asserts=False, num_devices=n_cores)
    xf_in = {}
    if profile_single_core:
        GDT0 = F32 if gather_dtype == "float32" else BF16
        xf_in["xfull1"] = nc.dram_tensor("xfull1_in", [N, 128], GDT0,
                                         kind="ExternalInput")
        xf_in["xfull2"] = nc.dram_tensor("xfull2_in", [N, 128], GDT0,
                                         kind="ExternalInput")

    # ---- I/O -------------------------------------------------------------
    tweetT_d = nc.dram_tensor("tweetT", [T, npc], GDT, kind="ExternalInput")
    idx_d = nc.dram_tensor("idx16", [128, slots * 8], I16,
                           kind="ExternalInput")
    keym_d = nc.dram_tensor("keym", [128, slots], F32, kind="ExternalInput")
    wgtm_d = nc.dram_tensor("wgtm", [128, slots], F32, kind="ExternalInput")
    Wt_d = nc.dram_tensor("Wt", [T, D], GDT, kind="ExternalInput")
    Wi_d = nc.dram_tensor("Wi", [D, D], F32, kind="ExternalInput")
    Wr0_d = nc.dram_tensor("Wr0", [D, D], F32, kind="ExternalInput")
    Wr1_d = nc.dram_tensor("Wr1", [D, D], F32, kind="ExternalInput")
    Wroot_d = nc.dram_tensor("Wroot", [D, D], F32, kind="ExternalInput")
    Wo_d = nc.dram_tensor("Wo", [D, D], F32, kind="ExternalInput")
    Wout_d = nc.dram_tensor("Wout", [D, 2], F32, kind="ExternalInput")
    bt_d = nc.dram_tensor("bt", [D, 1], F32, kind="ExternalInput")
    bi_d = nc.dram_tensor("bi", [D, 1], F32, kind="ExternalInput")
    brgcn_d = nc.dram_tensor("brgcn", [D, 1], F32, kind="ExternalInput")
    bo_d = nc.dram_tensor("bo", [D, 1], F32, kind="ExternalInput")
    bout_d = nc.dram_tensor("bout", [2, 1], F32, kind="ExternalInput")
    outT_d = nc.dram_tensor("outT", [2, npc], F32, kind="ExternalOutput")
    dbg = {}
    if debug_outputs:
        for name in ("x1T", "x2T", "x3T"):
            dbg[name] = nc.dram_tensor(name, [D, npc], F32,
                                       kind="ExternalOutput")
        dbg["xfull1"] = nc.dram_tensor("xfull1_out", [N, 128], GDT,
                                       kind="ExternalOutput")

    rg = [list(range(n_cores))]

    with tile.TileContext(nc) as tc, ExitStack() as ctx:
        # ---- persistent SBUF state --------------------------------------
        wpool = ctx.enter_context(tc.tile_pool(name="wpool", bufs=1))
        wt_sb = wpool.tile([128, KT * 128], GDT)
        for k in range(KT):
            nc.sync.dma_start(out=wt_sb[:, k * 128:(k + 1) * 128],
                              in_=Wt_d[k * 128:(k + 1) * 128, :])
        wi_sb = wpool.tile([128, 128], F32)
        nc.sync.dma_start(out=wi_sb[:], in_=Wi_d[:, :])
        wr0_sb = wpool.tile([128, 128], F32)
        nc.sync.dma_start(out=wr0_sb[:], in_=Wr0_d[:, :])
        wr1_sb = wpool.tile([128, 128], F32)
        nc.sync.dma_start(out=wr1_sb[:], in_=Wr1_d[:, :])
        wroot_sb = wpool.tile([128, 128], F32)
        nc.sync.dma_start(out=wroot_sb[:], in_=Wroot_d[:, :])
        wo_sb = wpool.tile([128, 128], F32)
        nc.sync.dma_start(out=wo_sb[:], in_=Wo_d[:, :])
        wout_sb = wpool.tile([128, 2], F32)
        nc.sync.dma_start(out=wout_sb[:], in_=Wout_d[:, :])
        bt_sb = wpool.tile([128, 1], F32)
        nc.sync.dma_start(out=bt_sb[:], in_=bt_d[:, :])
        bi_sb = wpool.tile([128, 1], F32)
        nc.sync.dma_start(out=bi_sb[:], in_=bi_d[:, :])
        brgcn_sb = wpool.tile([128, 1], F32)
        nc.sync.dma_start(out=brgcn_sb[:], in_=brgcn_d[:, :])
        bo_sb = wpool.tile([128, 1], F32)
        nc.sync.dma_start(out=bo_sb[:], in_=bo_d[:, :])
        bout_sb = wpool.tile([2, 1], F32)
        nc.sync.dma_start(out=bout_sb[:], in_=bout_d[:, :])

        idx_sb = wpool.tile([128, slots * 8], I16)
        nc.sync.dma_start(out=idx_sb[:], in_=idx_d[:, :])
        keym_sb = wpool.tile([128, slots], F32)
        nc.sync.dma_start(out=keym_sb[:], in_=keym_d[:, :])
        wgtm_sb = wpool.tile([128, slots], F32)
        nc.sync.dma_start(out=wgtm_sb[:], in_=wgtm_d[:, :])

        iota_sb = wpool.tile([128, 512], I16)
        nc.gpsimd.iota(iota_sb[:], pattern=[[1, 512]], base=0,
                       channel_multiplier=0,
                       allow_small_or_imprecise_dtypes=True)
        ident_sb = wpool.tile([128, 128], F32)
        make_identity(nc, ident_sb[:])

        # persistent transposed activations (xa reused for layer-2 output)
        xa = wpool.tile([128, npad], F32)   # x1T, later x3T
        xb = wpool.tile([128, npad], F32)   # x2T
        if npad > npc:
            nc.vector.memset(xa[:, npc:npad], 0.0)
            nc.vector.memset(xb[:, npc:npad], 0.0)

        # gather staging, manually rotated
        max_sg = max(sg_sizes)
        N_STAG = 4
        stag = [wpool.tile([128, NQ * max_sg * kq * 128], GDT,
                           name=f"stag{i}") for i in range(N_STAG)]

        # DRAM tiles for the all-gathers
        dpool = ctx.enter_context(tc.tile_pool(name="dpool", bufs=1,
                                               space="DRAM"))
        ag1_in = dpool.tile([npc, 128], GDT)
        ag2_in = dpool.tile([npc, 128], GDT)
        if profile_single_core:
            xfull1 = xf_in["xfull1"]
            xfull2 = xf_in["xfull2"]
        else:
            xfull1 = dpool.tile([N, 128], GDT)
            xfull2 = dpool.tile([N, 128], GDT)

        # ---- helpers ----------------------------------------------------
        def leaky_inplace(ap):
            # x = max(SLOPE * x, x)
            nc.vector.scalar_tensor_tensor(out=ap, in0=ap, scalar=SLOPE,
                                           in1=ap, op0=mybir.AluOpType.mult,
                                           op1=mybir.AluOpType.max)

        def transpose_to_nat(src_slice, w, nat_pool, tp_pool, ag_in):
            # src_slice: [128 feat, 128 nodes] slice of an xT tile
            ptp = tp_pool.tile([128, 128], F32, name="ptp")
            nc.tensor.transpose(ptp[:], src_slice, ident_sb[:])
            nat = nat_pool.tile([128, 128], GDT, name="nat")
            nc.vector.tensor_copy(out=nat[:], in_=ptp[:])
            wsz = min(128, npc - w * 128)
            nc.sync.dma_start(out=ag_in[w * 128: w * 128 + wsz, :],
                              in_=nat[:wsz, :])

        # ---- stage 1: x1 = leaky(tweet @ Wt + bt); leaky(x1 @ Wi + bi) --
        with tc.tile_pool(name="s1psum", bufs=2, space="PSUM") as s1psum, \
             tc.tile_pool(name="s1psum2", bufs=2, space="PSUM") as s1psum2, \
             tc.tile_pool(name="s1buf", bufs=3) as s1buf, \
             tc.tile_pool(name="s1nat", bufs=3) as s1nat, \
             tc.tile_pool(name="s1tp", bufs=2, space="PSUM") as s1tp:
            for t in range(NT):
                c0 = t * TILE_W
                cw = min(TILE_W, npc - c0)
                ps1 = s1psum.tile([128, TILE_W], F32, name="ps1")
                for k in range(KT):
                    tw = s1buf.tile([128, TILE_W], GDT, name="tw")
                    nc.sync.dma_start(
                        out=tw[:, :cw],
                        in_=tweetT_d[k * 128:(k + 1) * 128, c0:c0 + cw])
                    nc.tensor.matmul(ps1[:, :cw],
                                     lhsT=wt_sb[:, k * 128:(k + 1) * 128],
                                     rhs=tw[:, :cw],
                                     start=(k == 0), stop=(k == KT - 1))
                x1b = s1buf.tile([128, TILE_W], F32, name="x1b")
                nc.vector.tensor_scalar(out=x1b[:, :cw], in0=ps1[:, :cw],
                                        scalar1=bt_sb[:, :1], scalar2=None,
                                        op0=mybir.AluOpType.add)
                leaky_inplace(x1b[:, :cw])
                ps2 = s1psum2.tile([128, TILE_W], F32, name="ps2")
                nc.tensor.matmul(ps2[:, :cw], lhsT=wi_sb[:], rhs=x1b[:, :cw],
                                 start=True, stop=True)
                nc.vector.tensor_scalar(out=xa[:, c0:c0 + cw],
                                        in0=ps2[:, :cw],
                                        scalar1=bi_sb[:, :1], scalar2=None,
                                        op0=mybir.AluOpType.add)
                leaky_inplace(xa[:, c0:c0 + cw])
                for wi_ in range(c0 // 128, (c0 + cw + 127) // 128):
                    transpose_to_nat(xa[:, wi_ * 128:(wi_ + 1) * 128], wi_,
                                     s1nat, s1tp, ag1_in)

        if debug_outputs:
            nc.sync.dma_start(out=dbg["x1T"][:, :], in_=xa[:, :npc])

        if not profile_single_core:
            nc.gpsimd.collective_compute(
                "AllGather", mybir.AluOpType.bypass, replica_groups=rg,
                ins=[ag1_in.opt()], outs=[xfull1.opt()])

        if debug_outputs:
            nc.gpsimd.dma_start(out=dbg["xfull1"][:, :], in_=xfull1[:, :])

        # ---- RGCN layers -------------------------------------------------
        def rgcn_layer(xin, xout, xfull, ag_next):
            with tc.tile_pool(name="agg", bufs=4, space="PSUM") as aggp, \
                 tc.tile_pool(name="trp", bufs=2, space="PSUM") as trp, \
                 tc.tile_pool(name="tpp", bufs=2, space="PSUM") as tpp, \
                 tc.tile_pool(name="mp", bufs=12) as mp, \
                 tc.tile_pool(name="meanp", bufs=3) as meanp, \
                 tc.tile_pool(name="natp", bufs=3) as natp:
                sg_base = 0
                for g in range(groups):
                    S = sg_sizes[g]
                    st = stag[g % N_STAG]
                    for q in range(NQ):
                        n_i = S * kq * 128
                        off_i = (sg_base + q * S * kq) * 128
                        r0 = q * qrows
                        r1 = min(N, r0 + qrows)
                        nc.gpsimd.dma_gather(
                            out_ap=st[:, q * S * kq * 128:
                                      (q + 1) * S * kq * 128].rearrange(
                                          "p (c d) -> p c d", d=128),
                            in_ap=xfull[r0:r1, :],
                            idxs_ap=idx_sb[:, off_i // 16:
                                           (off_i + n_i) // 16],
                            num_idxs=n_i,
                            num_idxs_reg=n_i,
                            elem_size=128,
                        )
                    for pl in range(S):
                        p = g * prep["sgp"] + pl
                        pagg = aggp.tile([128, 512], F32, name="pagg")
                        nmm = NQ * kq
                        i_mm = 0
                        for q in range(NQ):
                            for k in range(kq):
                                slot = sg_base + (q * S + pl) * kq + k
                                sl = slot - sg_base
                                m = mp.tile([128, 512], GDT, name="m")
                                nc.vector.tensor_scalar(
                                    out=m[:], in0=iota_sb[:],
                                    scalar1=keym_sb[:, slot:slot + 1],
                                    scalar2=wgtm_sb[:, slot:slot + 1],
                                    op0=mybir.AluOpType.is_equal,
                                    op1=mybir.AluOpType.mult)
                                nc.tensor.matmul(
                                    pagg[:],
                                    lhsT=st[:, sl * 128:(sl + 1) * 128],
                                    rhs=m[:],
                                    start=(i_mm == 0), stop=(i_mm == nmm - 1))
                                i_mm += 1
                        mean = meanp.tile([128, 512], F32, name="mean")
                        nc.vector.tensor_copy(out=mean[:], in_=pagg[:])
                        ptr = trp.tile([128, 256], F32, name="ptr")
                        for wh in range(2):
                            w = p * 2 + wh
                            if w >= nw:
                                nc.vector.memset(ptr[:, wh * 128:
                                                     (wh + 1) * 128], 0.0)
                                continue
                            po = ptr[:, wh * 128:(wh + 1) * 128]
                            nc.tensor.matmul(
                                po, lhsT=wr0_sb[:],
                                rhs=mean[:, wh * 256:wh * 256 + 128],
                                start=True, stop=False)
                            nc.tensor.matmul(
                                po, lhsT=wr1_sb[:],
                                rhs=mean[:, wh * 256 + 128:wh * 256 + 256],
                                start=False, stop=False)
                            nc.tensor.matmul(
                                po, lhsT=wroot_sb[:],
                                rhs=xin[:, w * 128:(w + 1) * 128],
                                start=False, stop=True)
                        psz = min(256, npad - p * 256)
                        nc.vector.tensor_scalar(
                            out=xout[:, p * 256:p * 256 + psz],
                            in0=ptr[:, :psz], scalar1=brgcn_sb[:, :1],
                            scalar2=None, op0=mybir.AluOpType.add)
                        if ag_next is not None:
                            for wh in range(2):
                                w = p * 2 + wh
                                if w >= nw:
                                    continue
                                transpose_to_nat(
                                    xout[:, w * 128:(w + 1) * 128], w,
                                    natp, tpp, ag_next)
                    sg_base += S * NQ * kq

        rgcn_layer(xa, xb, xfull1, ag2_in)
        if debug_outputs:
            nc.sync.dma_start(out=dbg["x2T"][:, :], in_=xb[:, :npc])
        if not profile_single_core:
            nc.gpsimd.collective_compute(
                "AllGather", mybir.AluOpType.bypass, replica_groups=rg,
                ins=[ag2_in.opt()], outs=[xfull2.opt()])
        rgcn_layer(xb, xa, xfull2, None)
        if debug_outputs:
            nc.sync.dma_start(out=dbg["x3T"][:, :], in_=xa[:, :npc])

        # ---- head: leaky(x @ Wo + bo) @ Wout + bout ---------------------
        with tc.tile_pool(name="hps", bufs=2, space="PSUM") as hps, \
             tc.tile_pool(name="hps2", bufs=2, space="PSUM") as hps2, \
             tc.tile_pool(name="hbuf", bufs=3) as hbuf:
            for t in range(NT):
                c0 = t * TILE_W
                cw = min(TILE_W, npc - c0)
                psh = hps.tile([128, TILE_W], F32, name="psh")
                nc.tensor.matmul(psh[:, :cw], lhsT=wo_sb[:],
                                 rhs=xa[:, c0:c0 + cw], start=True, stop=True)
                hb = hbuf.tile([128, TILE_W], F32, name="hb")
                nc.vector.tensor_scalar(out=hb[:, :cw], in0=psh[:, :cw],
                                        scalar1=bo_sb[:, :1], scalar2=None,
                                        op0=mybir.AluOpType.add)
                leaky_inplace(hb[:, :cw])
                pso = hps2.tile([2, TILE_W], F32, name="pso")
                nc.tensor.matmul(pso[:, :cw], lhsT=wout_sb[:],
                                 rhs=hb[:, :cw], start=True, stop=True)
                ob = hbuf.tile([2, TILE_W], F32, name="ob")
                nc.vector.tensor_scalar(out=ob[:, :cw], in0=pso[:, :cw],
                                        scalar1=bout_sb[:, :1], scalar2=None,
                                        op0=mybir.AluOpType.add)
                nc.sync.dma_start(out=outT_d[:, c0:c0 + cw], in_=ob[:, :cw])

    nc.compile()
    return nc


# ---------------------------------------------------------------------------
# Public entry point
# ---------------------------------------------------------------------------

_CACHE = {}
GATHER_DTYPE = "bfloat16"


def _get_module(N, T, prep, gather_dtype=None, debug_outputs=False):
    if gather_dtype is None:
        gather_dtype = GATHER_DTYPE
    key = (N, T, prep["npc"], prep["kq"], prep["slots"], gather_dtype,
           debug_outputs)
    if key not in _CACHE:
        _CACHE[key] = _build_module(N, T, prep, gather_dtype=gather_dtype,
                                    debug_outputs=debug_outputs)
    return _CACHE[key]


def _make_in_maps(tweet, prep, Wt, bt, Wi, bi, Wrel, Wroot, brgcn, Wo, bo,
                  Wout, bout, n_cores=N_CORES, gather_dtype=None):
    import ml_dtypes
    if gather_dtype is None:
        gather_dtype = GATHER_DTYPE
    gdt = np.float32 if gather_dtype == "float32" else ml_dtypes.bfloat16
    npc = prep["npc"]
    f32 = np.float32
    shared = dict(
        Wt=np.ascontiguousarray(np.asarray(Wt, f32).astype(gdt)),
        Wi=np.ascontiguousarray(Wi, f32),
        Wr0=np.ascontiguousarray(Wrel[0], f32),
        Wr1=np.ascontiguousarray(Wrel[1], f32),
        Wroot=np.ascontiguousarray(Wroot, f32),
        Wo=np.ascontiguousarray(Wo, f32),
        Wout=np.ascontiguousarray(Wout, f32),
        bt=np.ascontiguousarray(np.reshape(bt, (-1, 1)), f32),
        bi=np.ascontiguousarray(np.reshape(bi, (-1, 1)), f32),
        brgcn=np.ascontiguousarray(np.reshape(brgcn, (-1, 1)), f32),
        bo=np.ascontiguousarray(np.reshape(bo, (-1, 1)), f32),
        bout=np.ascontiguousarray(np.reshape(bout, (-1, 1)), f32),
    )
    in_maps = []
    for c in range(n_cores):
        m = dict(shared)
        m["tweetT"] = np.ascontiguousarray(
            tweet[c * npc:(c + 1) * npc].T.astype(gdt))
        m["idx16"] = np.ascontiguousarray(prep["idx16"][c])
        m["keym"] = np.ascontiguousarray(prep["keym"][c])
        m["wgtm"] = np.ascontiguousarray(prep["wgtm"][c])
        in_maps.append(m)
    return in_maps


def kernel(tweet, edge_index, edge_type, Wt, bt, Wi, bi, Wrel, Wroot, brgcn,
           Wo, bo, Wout, bout):
    tweet = np.asarray(tweet, dtype=np.float32)
    N, T = tweet.shape
    prep = _preprocess(edge_index, edge_type, N)
    nc = _get_module(N, T, prep)
    in_maps = _make_in_maps(tweet, prep, Wt, bt, Wi, bi, Wrel, Wroot, brgcn,
                            Wo, bo, Wout, bout)
    res = bass_utils.run_bass_kernel_spmd(
        nc, in_maps, core_ids=list(range(N_CORES)))
    out = np.concatenate(
        [res.results[c]["outT"].T for c in range(N_CORES)], axis=0)
    return np.ascontiguousarray(out, dtype=np.float32)
